# revision 26
# baseline (speedup 1.0000x reference)
"""GQA attention (RoPE, causal) for one TRN2 chip (8 NeuronCores).

Sharding: core d handles batch b = d//4 and kv-group g = d%4
(8 q heads + 1 kv head per core).  Each core computes its partial
output-projection contribution out_partial[b] (shape [S, H]); the host
sums the 4 partials per batch and adds bo.  No collectives.

Layout strategy (per core, all matmul contractions on partitions):
  xT  [H, S]      : x[b] transposed on host, bf16
  QT  [512, S]    : q^T head-major (rope'd), from lhsT=wqT, rhs=xT
  KT2 [128, S]    : k^T rope'd, duplicated in both partition halves
  V   [S, 64]+1s  : v with an appended ones column (denominator trick)
  scores^T [sj,si]: lhsT=KT2 chunk, rhs=QT chunk  (no transposes needed)
  exp (no max-subtraction; |scores/8| <~ 6 so exp is safe in fp32/bf16)
  PV: lhsT=[V|1] [sj,65], rhs=expS^T -> psum [65, si] = [attn^T; denom]
  out[s,o]: lhsT=attnT chunk, rhs=woT chunk, accumulated over m tiles.
"""

import sys

if "/opt/trn_rl_repo" not in sys.path:
    sys.path.insert(0, "/opt/trn_rl_repo")

import numpy as np
import ml_dtypes

bf16 = ml_dtypes.bfloat16

B = 2
S = 2048
H = 2048
N_HEADS = 32
KV_HEADS = 4
HEAD_DIM = 64
ROPE_THETA = 10000.0
N_CORES = 8
ML = 512          # q-head features per core (8 heads * 64)
CHUNK = 512       # si chunk width
SJB = 128         # sj block width
HB = 128          # h (contraction) tile


def build_graph(nc, tile_mod, mybir, seq=S):
    """Emit the per-core graph. seq can be shrunk for simulator tests."""
    fp32 = mybir.dt.float32
    bfl = mybir.dt.bfloat16

    nC = seq // CHUNK       # si chunks
    nJ = seq // SJB         # sj blocks
    nHB = H // HB           # contraction tiles
    nMT = ML // 128         # q-feature partition tiles (head pairs)

    xT = nc.dram_tensor("xT", [H, seq], bfl, kind="ExternalInput")
    wq = nc.dram_tensor("wq", [H, ML], bfl, kind="ExternalInput")
    wkv = nc.dram_tensor("wkv", [H, 128], bfl, kind="ExternalInput")
    wo = nc.dram_tensor("wo", [ML, H], bfl, kind="ExternalInput")
    bq = nc.dram_tensor("bq", [128, nMT], fp32, kind="ExternalInput")
    bkv = nc.dram_tensor("bkv", [128, 1], fp32, kind="ExternalInput")
    cos2 = nc.dram_tensor("cos2", [128, seq], bfl, kind="ExternalInput")
    sinS = nc.dram_tensor("sinS", [128, seq], bfl, kind="ExternalInput")
    maskt = nc.dram_tensor("maskt", [128, 4 * CHUNK], bfl, kind="ExternalInput")
    id64 = nc.dram_tensor("id64", [64, 64], bfl, kind="ExternalInput")
    ones64 = nc.dram_tensor("ones64", [1, 64], bfl, kind="ExternalInput")
    out = nc.dram_tensor("out", [seq, H], fp32, kind="ExternalOutput")

    Exp = mybir.ActivationFunctionType.Exp
    tc = tile_mod.TileContext(nc)
    with tc:
        with tc.tile_pool(name="persist", bufs=1) as P:
            wo_t = [P.tile([128, H], bfl, tag=f"wo{mt}", name=f"wo{mt}")
                    for mt in range(nMT)]
            qt = [P.tile([128, seq], bfl, tag=f"qt{mt}", name=f"qt{mt}")
                  for mt in range(nMT)]
            at = [P.tile([128, seq], bfl, tag=f"at{mt}", name=f"at{mt}")
                  for mt in range(nMT)]
            kt2 = P.tile([128, seq], bfl, tag="kt2", name="kt2")
            mask_t = P.tile([128, 4 * CHUNK], bfl, tag="mask", name="mask_t")
            vones = [P.tile([128, 65], bfl, tag=f"vo{j}", name=f"vo{j}")
                     for j in range(nJ)]

            ones64_t = P.tile([1, 64], bfl, tag="ones64", name="ones64_t")
            for mt in range(nMT):
                nc.sync.dma_start(wo_t[mt][:], wo.ap()[128 * mt:128 * mt + 128, :])
            nc.sync.dma_start(mask_t[:], maskt.ap())
            nc.sync.dma_start(ones64_t[:], ones64.ap())

            # ---------------- phase 1: projections -------------------------
            with tc.tile_pool(name="ph1", bufs=1) as Bp, \
                 tc.tile_pool(name="ph1tmp", bufs=6) as T, \
                 tc.tile_pool(name="ph1ps", bufs=3, space="PSUM") as PS1, \
                 tc.tile_pool(name="vtps", bufs=2, space="PSUM") as PSV:
                xt = [Bp.tile([128, seq], bfl, tag=f"x{hb}", name=f"x{hb}")
                      for hb in range(nHB)]
                wq_t = [Bp.tile([128, ML], bfl, tag=f"wq{hb}", name=f"wq{hb}")
                        for hb in range(nHB)]
                wkv_t = [Bp.tile([128, 128], bfl, tag=f"wkv{hb}", name=f"wkv{hb}")
                         for hb in range(nHB)]
                cos_t = Bp.tile([128, seq], bfl, tag="cos", name="cos_t")
                sin_t = Bp.tile([128, seq], bfl, tag="sin", name="sin_t")
                bq_t = Bp.tile([128, nMT], fp32, tag="bq", name="bq_t")
                bkv_t = Bp.tile([128, 1], fp32, tag="bkv", name="bkv_t")
                id64_t = Bp.tile([64, 64], bfl, tag="id64", name="id64_t")
                vt_sb = Bp.tile([64, seq], bfl, tag="vt", name="vt_sb")

                for hb in range(nHB):
                    nc.sync.dma_start(wq_t[hb][:], wq.ap()[128 * hb:128 * hb + 128, :])
                    nc.sync.dma_start(wkv_t[hb][:], wkv.ap()[128 * hb:128 * hb + 128, :])
                    nc.sync.dma_start(xt[hb][:], xT.ap()[128 * hb:128 * hb + 128, :])
                nc.sync.dma_start(cos_t[:], cos2.ap())
                nc.sync.dma_start(sin_t[:], sinS.ap())
                nc.sync.dma_start(bq_t[:], bq.ap())
                nc.sync.dma_start(bkv_t[:], bkv.ap())
                nc.sync.dma_start(id64_t[:], id64.ap())

                def rope_group(ps, bias_col, cs, dsts):
                    """RoPE a [n,CHUNK] psum group (n = 64*len(dsts)) into
                    per-64-row destinations.  All DVE ops keep both SBUF
                    inputs at equal base partitions (HW requirement); the
                    half-swap uses single-input shifted copies."""
                    n = 64 * len(dsts)
                    t0 = T.tile([128, CHUNK], bfl, tag="t0", name="t0")
                    nc.vector.tensor_scalar_add(t0[0:n, :], ps[0:n, :],
                                                bias_col[0:n, :])
                    t1 = T.tile([128, CHUNK], bfl, tag="t1", name="t1")
                    nc.vector.tensor_mul(t1[0:n, :], t0[0:n, :], cos_t[0:n, cs])
                    rs = T.tile([128, CHUNK], bfl, tag="rs", name="rs")
                    nc.vector.tensor_mul(rs[0:n, :], t0[0:n, :], sin_t[0:n, cs])
                    r2 = T.tile([128, CHUNK], bfl, tag="r2", name="r2")
                    for b in range(len(dsts)):
                        nc.vector.tensor_copy(r2[64 * b:64 * b + 32, :],
                                              rs[64 * b + 32:64 * b + 64, :])
                        nc.vector.tensor_copy(r2[64 * b + 32:64 * b + 64, :],
                                              rs[64 * b:64 * b + 32, :])
                    for b, dst in enumerate(dsts):
                        nc.vector.tensor_add(dst, t1[64 * b:64 * b + 64, :],
                                             r2[64 * b:64 * b + 64, :])
                    return t0

                # KV projection (+ rope K, copy V)
                for c in range(nC):
                    cs = slice(CHUNK * c, CHUNK * (c + 1))
                    ps = PS1.tile([128, CHUNK], fp32, tag="pskv", name="pskv")
                    for hb in range(nHB):
                        nc.tensor.matmul(ps[:], wkv_t[hb][:], xt[hb][:, cs],
                                         start=(hb == 0), stop=(hb == nHB - 1))
                    t0 = rope_group(ps, bkv_t[:, 0:1], cs, [kt2[0:64, cs]])
                    nc.vector.tensor_scalar_add(t0[64:128, :], ps[64:128, :],
                                                bkv_t[64:128, 0:1])
                    nc.vector.tensor_copy(kt2[64:128, cs], kt2[0:64, cs])
                    nc.vector.tensor_copy(vt_sb[:, cs], t0[64:128, :])

                # V transposes into [sj, 64]+ones tiles
                for j in range(nJ):
                    pv = PSV.tile([128, 64], bfl, tag="psv", name="psv")
                    nc.tensor.transpose(pv[:], vt_sb[:, 128 * j:128 * j + 128],
                                        id64_t[:])
                    nc.vector.memset(vones[j][:, 64:65], 1.0)
                    nc.vector.tensor_copy(vones[j][:, 0:64], pv[:])

                # Q projection (+ rope)
                for mt in range(nMT):
                    ms = slice(128 * mt, 128 * mt + 128)
                    for c in range(nC):
                        cs = slice(CHUNK * c, CHUNK * (c + 1))
                        ps = PS1.tile([128, CHUNK], fp32, tag="psq", name="psq")
                        for hb in range(nHB):
                            nc.tensor.matmul(ps[:], wq_t[hb][:, ms], xt[hb][:, cs],
                                             start=(hb == 0), stop=(hb == nHB - 1))
                        rope_group(ps, bq_t[:, mt:mt + 1], cs,
                                   [qt[mt][0:64, cs], qt[mt][64:128, cs]])

            # ---------------- phase 2+3: attention + out-proj ---------------
            with tc.tile_pool(name="qkps", bufs=2, space="PSUM") as QKP, \
                 tc.tile_pool(name="pvps", bufs=2, space="PSUM") as PVP, \
                 tc.tile_pool(name="ops", bufs=2, space="PSUM") as OPS, \
                 tc.tile_pool(name="expp", bufs=18) as EP, \
                 tc.tile_pool(name="small", bufs=8) as SM, \
                 tc.tile_pool(name="outb", bufs=2) as OB:
                for c in range(nC):
                    cs = slice(CHUNK * c, CHUNK * (c + 1))
                    njb = 4 * c + 4
                    for mt in range(nMT):
                        pv0 = PVP.tile([65, CHUNK], fp32, tag="pv", name="pv0")
                        pv1 = PVP.tile([65, CHUNK], fp32, tag="pv", name="pv1")
                        # All QK matmuls + Exp first (both heads interleaved:
                        # the two heads sit in partition halves, so walrus
                        # row-tiles them onto disjoint PE quadrants), then all
                        # PV matmuls — avoids PE tiling-mode thrash.
                        # sj blocks go in pairs so each Exp covers 1024 cols
                        # (amortizes ACT's ~352-cycle fixed overhead).
                        ews = [[], []]
                        for p in range(njb // 2):
                            jb0, jb1 = 2 * p, 2 * p + 1
                            for hh, pbase in ((0, 0), (1, 64)):
                                qsl = slice(pbase, pbase + 64)
                                qw = QKP.tile([128, 2 * CHUNK], fp32, tag="qk",
                                              name="qw")
                                for i, jb in enumerate((jb0, jb1)):
                                    js = slice(128 * jb, 128 * jb + 128)
                                    nc.tensor.matmul(
                                        qw[:, CHUNK * i:CHUNK * (i + 1)],
                                        kt2[qsl, js], qt[mt][qsl, cs],
                                        start=True, stop=True)
                                ew = EP.tile([128, 2 * CHUNK], bfl, tag="e",
                                             name="ew")
                                nc.scalar.activation(ew[:], qw[:], Exp, scale=0.125)
                                rel = p - 2 * c
                                if rel >= 0:
                                    msl = slice(2 * CHUNK * rel,
                                                2 * CHUNK * (rel + 1))
                                    nc.vector.tensor_mul(ew[:], ew[:],
                                                         mask_t[:, msl])
                                ews[hh].append(ew)
                        for hh, pvt in ((0, pv0), (1, pv1)):
                            for p in range(njb // 2):
                                ew = ews[hh][p]
                                for i, jb in enumerate((2 * p, 2 * p + 1)):
                                    nc.tensor.matmul(
                                        pvt[:], vones[jb][:, 0:65],
                                        ew[:, CHUNK * i:CHUNK * (i + 1)],
                                        start=(jb == 0), stop=(jb == njb - 1))
                        for hh, pv in ((0, pv0), (1, pv1)):
                            rf = SM.tile([1, CHUNK], fp32, tag="rf", name="rf")
                            nc.vector.reciprocal(rf[:], pv[64:65, :])
                            rb = SM.tile([1, CHUNK], bfl, tag="rb", name="rb")
                            nc.vector.tensor_copy(rb[:], rf[:])
                            bc = QKP.tile([64, CHUNK], fp32, tag="qk", name="bc")
                            nc.tensor.matmul(bc[:], ones64_t[:], rb[:],
                                             start=True, stop=True)
                            bcs = SM.tile([64, CHUNK], bfl, tag="bcs", name="bcs")
                            nc.scalar.copy(bcs[:], bc[:])
                            nc.vector.tensor_mul(at[mt][64 * hh:64 * hh + 64, cs],
                                                 pv[0:64, :], bcs[:])
                    # out-projection for this chunk
                    for st in range(4):
                        sit = 4 * c + st
                        ss = slice(128 * sit, 128 * sit + 128)
                        ob = OB.tile([128, H], fp32, tag="ob", name="ob")
                        for oc in range(4):
                            po = OPS.tile([128, CHUNK], fp32, tag="po", name="po")
                            for mt in range(nMT):
                                nc.tensor.matmul(po[:], at[mt][:, ss],
                                                 wo_t[mt][:, CHUNK * oc:CHUNK * (oc + 1)],
                                                 start=(mt == 0), stop=(mt == nMT - 1))
                            nc.vector.tensor_copy(
                                ob[:, CHUNK * oc:CHUNK * (oc + 1)], po[:])
                        nc.sync.dma_start(out.ap()[ss, :], ob[:])
    return nc


# ---------------------------------------------------------------------------
# host side
# ---------------------------------------------------------------------------

def _rope_tables(seq):
    inv_freq = 1.0 / (ROPE_THETA ** (np.arange(0, HEAD_DIM, 2, dtype=np.float32)
                                     / HEAD_DIM))
    t = np.arange(seq, dtype=np.float32)
    freqs = np.outer(t, inv_freq)                       # [S, 32]
    emb = np.concatenate([freqs, freqs], axis=-1)       # [S, 64]
    cos_t = np.cos(emb).astype(np.float32)
    sin_t = np.sin(emb).astype(np.float32)
    cos2 = np.tile(cos_t.T, (2, 1)).copy()              # [128, S]
    # "shuffled" sign layout: row r holds the multiplier that, after the
    # half-swap copy (rows r <-> r^32 within each 64-block), lands the
    # correct signed sin on the output row: +sin for r%64<32, -sin above.
    sgn = np.where(np.arange(HEAD_DIM) < HEAD_DIM // 2, 1.0, -1.0).astype(np.float32)
    sinS = np.tile((sin_t * sgn).T, (2, 1)).copy()      # [128, S]
    return cos2, sinS


def _masks():
    r = np.arange(128)[:, None]
    cc = np.arange(CHUNK)[None, :]
    cols = []
    for o in range(4):
        cols.append((cc >= r + 128 * o).astype(np.float32))
    return np.concatenate(cols, axis=1).astype(bf16)    # [128, 2048]


def host_inputs(x, Wq, bq, Wk, bk, Wv, bv, Wo, seq=S):
    """Build in_maps for the 8 cores."""
    cos2, sinS = _rope_tables(seq)
    cos2 = cos2.astype(bf16)
    sinS = sinS.astype(bf16)
    maskt = _masks()
    id64 = np.eye(64, dtype=np.float32).astype(bf16)
    ones64 = np.ones((1, 64), dtype=np.float32).astype(bf16)
    xTb = [np.ascontiguousarray(x[b, :seq, :].T).astype(bf16) for b in range(B)]
    in_maps = []
    for d in range(N_CORES):
        b, g = d // 4, d % 4
        wq_s = np.ascontiguousarray(Wq[ML * g:ML * (g + 1), :].T).astype(bf16)
        wk_s = np.ascontiguousarray(Wk[64 * g:64 * (g + 1), :].T).astype(bf16)
        wv_s = np.ascontiguousarray(Wv[64 * g:64 * (g + 1), :].T).astype(bf16)
        wkv_s = np.concatenate([wk_s, wv_s], axis=1)
        wo_s = np.ascontiguousarray(Wo[:, ML * g:ML * (g + 1)].T).astype(bf16)
        bq_s = np.ascontiguousarray(
            bq[ML * g:ML * (g + 1)].reshape(4, 128).T).astype(np.float32)
        bkv_s = np.concatenate([bk[64 * g:64 * (g + 1)],
                                bv[64 * g:64 * (g + 1)]]).reshape(128, 1)
        in_maps.append({
            "xT": xTb[b], "wq": wq_s, "wkv": wkv_s, "wo": wo_s,
            "bq": bq_s, "bkv": np.ascontiguousarray(bkv_s, dtype=np.float32),
            "cos2": cos2[:, :seq], "sinS": sinS[:, :seq], "maskt": maskt,
            "id64": id64, "ones64": ones64,
        })
    return in_maps


_NC = None


def _get_nc():
    global _NC
    if _NC is None:
        import concourse.tile as tile_mod
        from concourse import bacc, mybir
        nc = bacc.Bacc("TRN2", target_bir_lowering=False, debug=False,
                       num_devices=N_CORES)
        build_graph(nc, tile_mod, mybir)
        nc.compile()
        _NC = nc
    return _NC


def kernel(**inputs):
    from concourse import bass_utils
    nc = _get_nc()
    x = np.asarray(inputs["x"], dtype=np.float32)
    in_maps = host_inputs(
        x, np.asarray(inputs["Wq"], np.float32), np.asarray(inputs["bq"], np.float32),
        np.asarray(inputs["Wk"], np.float32), np.asarray(inputs["bk"], np.float32),
        np.asarray(inputs["Wv"], np.float32), np.asarray(inputs["bv"], np.float32),
        np.asarray(inputs["Wo"], np.float32))
    res = bass_utils.run_bass_kernel_spmd(nc, in_maps, core_ids=list(range(N_CORES)))
    bo = np.asarray(inputs["bo"], np.float32)
    out = np.empty((B, S, H), dtype=np.float32)
    for b in range(B):
        acc = res.results[4 * b]["out"].astype(np.float32).copy()
        for g in range(1, 4):
            acc += res.results[4 * b + g]["out"]
        out[b] = acc + bo[None, :]
    return out
